# revision 3
# baseline (speedup 1.0000x reference)
"""Trainium2 Bass kernel for BinaryDecoderV2 — v2.5 (bit-packed weights + fp8 DoubleRow).

Computes loss = mean(((latent @ int_weights) - int_sum)^2 / 255^2) where
int_weights packs sign bits of `weight` into two's-complement ints and
int_sum packs `true_sum` the same way.

Sharding: tensor-parallel over out_features across 8 NeuronCores (each core
owns 128 of the 1024 outputs; latent replicated). No collectives — each core
emits a partial sum of squared diffs [128, 4]; the host reduces to the loss.

Per-core pipeline:
  - weights arrive as sign BITS, 8 per byte (w8[q, kt, o] bit b = sign of
    weight bit b for k=kt*128+q): 1MB instead of 8MB fp8 — DMA is the
    roofline here.
  - DVE unpack, 3 bitwise passes per byte: fp8e4 bit patterns 0x00-0x0F are
    LINEAR (value = n * 2^-9), so (x & 15), (x >> 4) & 15, (x >> 7) written
    raw into an fp8-typed tile give the radix-16 digits of int_w times 2^-9.
  - PE pack: diagonal matmuls with per-plane coefficients (-0.5, -8, +128;
    all fp8e4-exact) -> psum = -int_w/1024 exactly; planes (lo, hi) ride one
    DoubleRow matmul, the sign plane a regular one.
  - ACT cast: Copy(psum*1024) -> intw fp8e4 (= -int_w, RNE-rounded; error
    analysis gives ~6e-4 relative loss error, far under the 2e-2 gate).
  - int_sum: DoubleRow diagonal matmuls (+powers pairs) from fp8 true_sum
    planes accumulate +int_sum into the 4 main PSUM banks (warms the PE).
  - main matmuls: fp8e4 x fp8e4 DoubleRow, 256-deep contraction per MM:
    psum += intw.T @ latq = -pred  ->  psum = int_sum - pred = -diff.
  - loss: ACT Square+accum_out from PSUM -> partials [128, 4] per core.
  - head DMAs are chunked (w8 per mega-round, tq per plane-pair) so the PE
    and DVE start within ~2-3us instead of waiting for whole tensors.
"""

import numpy as np
import ml_dtypes

IN_FEATURES = 8192
OUT_FEATURES = 1024
N_BITS = 8
BATCH = 2048
N_CORES = 8
OPC = OUT_FEATURES // N_CORES  # 128 outputs per core
KP = 128                       # k per subtile (partition dim)
KT = IN_FEATURES // KP         # 64 k-subtiles
DKT = KT // 2                  # 32 DoubleRow k-tiles
NRND = 16                      # pack rounds (4 kt each)
MEGA = 4                       # unpack mega-rounds (4 rounds each)
NCHUNK = 512                   # moving free dim per matmul
NCH = BATCH // NCHUNK          # 4 batch chunks
# latent chunk schedule (kt per chunk), tapered tail
_LCH = [8, 8, 8, 8, 8, 8, 8, 4, 2, 2]
LCH_START = {}
_s = 0
for _i, _n in enumerate(_LCH):
    LCH_START[_s] = (_i, _n)
    _s += _n
assert _s == KT
POWERS = [1.0, 2.0, 4.0, 8.0, 16.0, 32.0, 64.0, -128.0]
# nibble-plane coefficients: psum = -int_w/1024, planes valued n*2^-9
C_PLANES = [-0.5, -8.0, 128.0]   # lo nibble, hi nibble, sign bit
SCALE = 2.0 ** N_BITS - 1.0

_CACHE: dict = {}


def _build():
    import concourse.bacc as bacc
    import concourse.mybir as mybir
    from concourse import tile

    f8e4 = mybir.dt.float8e4
    u8 = mybir.dt.uint8
    f32 = mybir.dt.float32
    Act = mybir.ActivationFunctionType
    Alu = mybir.AluOpType
    PM = mybir.MatmulPerfMode

    nc = bacc.Bacc("TRN2", target_bir_lowering=False, debug=False,
                   num_devices=N_CORES)

    latq = nc.dram_tensor("latq", [128, KT, BATCH], f8e4,
                          kind="ExternalInput")
    w8 = nc.dram_tensor("w8", [128, KT, OPC], u8, kind="ExternalInput")
    tq = nc.dram_tensor("tq", [OPC, N_BITS, BATCH], f8e4,
                        kind="ExternalInput")
    dg = nc.dram_tensor("dg", [OPC, N_BITS, OPC], f8e4,
                        kind="ExternalInput")
    cp = nc.dram_tensor("cp", [128, 3, 128], f8e4, kind="ExternalInput")
    partials = nc.dram_tensor("partials", [128, NCH], f32,
                              kind="ExternalOutput")

    with tile.TileContext(nc) as tc:
        with (
            tc.tile_pool(name="w8p", bufs=1) as w8_pool,
            tc.tile_pool(name="tsp", bufs=1) as tsp_pool,
            tc.tile_pool(name="cst", bufs=1) as cst_pool,
            tc.tile_pool(name="tp", bufs=2) as t_pool,
            tc.tile_pool(name="iw", bufs=1) as iw_pool,
            tc.tile_pool(name="lat", bufs=3) as lat_pool,
            tc.tile_pool(name="loss", bufs=1) as loss_pool,
            tc.tile_pool(name="ps", bufs=1, space="PSUM") as psum_pool,
            tc.tile_pool(name="pk", bufs=2, space="PSUM") as pk_pool,
        ):
            # ---- tiny constants first, then chunked heads ----
            dgt = cst_pool.tile([OPC, N_BITS, OPC], f8e4, name="dgt",
                                tag="dgt")
            nc.sync.dma_start(dgt[:], dg[:])
            cpt = cst_pool.tile([128, 3, 128], f8e4, name="cpt", tag="cpt")
            nc.sync.dma_start(cpt[:], cp[:])

            w8t = w8_pool.tile([128, KT, OPC], u8)
            tp = tsp_pool.tile([OPC, N_BITS, BATCH], f8e4)
            for h in range(4):
                nc.sync.dma_start(w8t[:, 16 * h:16 * (h + 1), :],
                                  w8[:, 16 * h:16 * (h + 1), :])
                nc.sync.dma_start(tp[:, 2 * h:2 * h + 2, :],
                                  tq[:, 2 * h:2 * h + 2, :])

            # ---- int_sum: psum[o, n] = +int_sum (DoubleRow diag MMs;
            # also warms the PE) ----
            psums = [psum_pool.tile([128, NCHUNK], f32, name=f"ps{i}",
                                    tag=f"ps{i}") for i in range(NCH)]
            for bp in range(4):
                for c in range(NCH):
                    nc.tensor.matmul(
                        psums[c][:],
                        dgt[:, 2 * bp:2 * bp + 2, :],
                        tp[:, 2 * bp:2 * bp + 2,
                           c * NCHUNK:(c + 1) * NCHUNK],
                        start=(bp == 0), stop=False,
                        perf_mode=PM.DoubleRow)

            # ---- weight pipeline + main matmul stream ----
            iwts = [iw_pool.tile([128, 4, OPC], f8e4, name=f"iw{r}",
                                 tag=f"iw{r}") for r in range(NRND)]
            out_t = loss_pool.tile([128, NCH], f32, name="out_t",
                                   tag="out_t")
            lts = {}

            def issue_lat(kt):
                if kt in LCH_START:
                    q, n = LCH_START[kt]
                    lt = lat_pool.tile([128, n, BATCH], f8e4,
                                       name=f"lt{q}", tag="lat")
                    nc.sync.dma_start(lt[:], latq[:, kt:kt + n, :])
                    lts[kt] = (lt, kt)

            for mr in range(MEGA):
                # unpack 16 kt worth: 3 nibble planes, t[q, pl, 16*OPC]
                tt = t_pool.tile([128, 3, 16 * OPC], f8e4,
                                 name=f"tt{mr}", tag="tt")
                w8s = w8t[:, 16 * mr:16 * (mr + 1), :]
                nc.vector.tensor_scalar(tt[:, 0, :].bitcast(u8), w8s,
                                        15, None, Alu.bitwise_and)
                nc.vector.tensor_scalar(tt[:, 1, :].bitcast(u8), w8s,
                                        4, 15, Alu.logical_shift_right,
                                        Alu.bitwise_and)
                nc.vector.tensor_scalar(tt[:, 2, :].bitcast(u8), w8s,
                                        7, None, Alu.logical_shift_right)

                for rr in range(4):          # rounds within mega-round
                    r = 4 * mr + rr
                    # pack: pkb = -int_w/1024 for 4 kt (512 cols)
                    pkb = pk_pool.tile([128, NCHUNK], f32, name=f"pk{r}",
                                       tag="pk")
                    nc.tensor.matmul(
                        pkb[:], cpt[:, 0:2, :],
                        tt[:, 0:2, rr * NCHUNK:(rr + 1) * NCHUNK],
                        start=True, stop=False, perf_mode=PM.DoubleRow)
                    nc.tensor.matmul(
                        pkb[:], cpt[:, 2, :],
                        tt[:, 2, rr * NCHUNK:(rr + 1) * NCHUNK],
                        start=False, stop=True)
                    # cast: intw = -int_w (fp8e4, RNE)
                    nc.scalar.activation(iwts[r][:], pkb[:], Act.Copy,
                                         scale=1024.0)
                    # main MMs for dkt = 2r, 2r+1
                    for i in range(2):
                        dkt = 2 * r + i
                        kt0 = 2 * dkt
                        issue_lat(kt0)
                        cur = lts[max(s for s in lts if s <= kt0)]
                        lt, base = cur
                        a = kt0 - base
                        lhsT = iwts[r][:, 2 * i:2 * i + 2, :]
                        last = (dkt == DKT - 1)
                        for c in range(NCH):
                            nc.tensor.matmul(
                                psums[c][:], lhsT,
                                lt[:, a:a + 2,
                                   c * NCHUNK:(c + 1) * NCHUNK],
                                start=False, stop=last,
                                perf_mode=PM.DoubleRow)
                            if last:
                                # interleave loss squares with final MMs
                                d2 = loss_pool.tile(
                                    [128, NCHUNK], f32, name=f"d2_{c}",
                                    tag=f"d2_{c}")
                                nc.scalar.activation(
                                    d2[:], psums[c][:], Act.Square,
                                    accum_out=out_t[:, c:c + 1])

            nc.sync.dma_start(partials[:], out_t[:])

    nc.compile()
    return nc


def _get_nc():
    if "nc" not in _CACHE:
        _CACHE["nc"] = _build()
    return _CACHE["nc"]


def make_in_maps(latent: np.ndarray, true_sum: np.ndarray,
                 weight: np.ndarray) -> list:
    f8 = ml_dtypes.float8_e4m3fn

    # latq[p, kt, n] = latent[n, kt*128 + p]
    lat8 = latent.astype(f8)
    latq = np.ascontiguousarray(
        lat8.T.reshape(KT, KP, BATCH).transpose(1, 0, 2))

    # sign bits, packed 8 per byte: byte[k, o] bit b = (weight[k, o*8+b] > 0)
    bits = (weight > 0).astype(np.uint8).reshape(IN_FEATURES,
                                                 OUT_FEATURES, N_BITS)
    shifts = (1 << np.arange(N_BITS, dtype=np.uint16))
    bytes_ko = (bits.astype(np.uint16) * shifts).sum(-1).astype(np.uint8)

    # diag pairs for int_sum: dg[o, b, o'] = powers[b] * (o == o')
    dg = np.zeros((OPC, N_BITS, OPC), dtype=np.float32)
    for b in range(N_BITS):
        np.fill_diagonal(dg[:, b, :], POWERS[b])
    dg8 = dg.astype(f8)

    # pack plane coefficients
    cpm = np.zeros((128, 3, 128), dtype=np.float32)
    for j in range(3):
        np.fill_diagonal(cpm[:, j, :], C_PLANES[j])
    cp8 = cpm.astype(f8)

    in_maps = []
    for c in range(N_CORES):
        # w8[q, kt, o] = bytes_ko[kt*128+q, c*128+o]
        wcol = bytes_ko[:, c * OPC:(c + 1) * OPC]
        w8 = np.ascontiguousarray(
            wcol.reshape(KT, KP, OPC).transpose(1, 0, 2))
        T = true_sum[:, c * OPC * N_BITS:(c + 1) * OPC * N_BITS]
        tql = np.ascontiguousarray(
            T.reshape(BATCH, OPC, N_BITS).transpose(1, 2, 0)).astype(f8)
        in_maps.append({"latq": latq, "w8": w8, "tq": tql,
                        "dg": dg8, "cp": cp8})
    return in_maps


def kernel(latent: np.ndarray, true_sum: np.ndarray,
           weight: np.ndarray) -> np.ndarray:
    from concourse.bass_utils import run_bass_kernel_spmd

    nc = _get_nc()
    in_maps = make_in_maps(latent, true_sum, weight)
    res = run_bass_kernel_spmd(nc, in_maps, list(range(N_CORES)))

    total = 0.0
    for c in range(N_CORES):
        total += float(res.results[c]["partials"].astype(np.float64).sum())
    loss = total / (BATCH * OUT_FEATURES) / (SCALE * SCALE)
    return np.array(loss, dtype=np.float32)


# revision 4
# speedup vs baseline: 1.0325x; 1.0325x over previous
"""Trainium2 Bass kernel for BinaryDecoderV2 — v2.5 (bit-packed weights + fp8 DoubleRow).

Computes loss = mean(((latent @ int_weights) - int_sum)^2 / 255^2) where
int_weights packs sign bits of `weight` into two's-complement ints and
int_sum packs `true_sum` the same way.

Sharding: tensor-parallel over out_features across 8 NeuronCores (each core
owns 128 of the 1024 outputs; latent replicated). No collectives — each core
emits a partial sum of squared diffs [128, 4]; the host reduces to the loss.

Per-core pipeline:
  - weights arrive as sign BITS, 8 per byte (w8[q, kt, o] bit b = sign of
    weight bit b for k=kt*128+q): 1MB instead of 8MB fp8 — DMA is the
    roofline here.
  - DVE unpack, 3 bitwise passes per byte: fp8e4 bit patterns 0x00-0x0F are
    LINEAR (value = n * 2^-9), so (x & 15), (x >> 4) & 15, (x >> 7) written
    raw into an fp8-typed tile give the radix-16 digits of int_w times 2^-9.
  - PE pack: diagonal matmuls with per-plane coefficients (-0.5, -8, +128;
    all fp8e4-exact) -> psum = -int_w/1024 exactly; planes (lo, hi) ride one
    DoubleRow matmul, the sign plane a regular one.
  - ACT cast: Copy(psum*1024) -> intw fp8e4 (= -int_w, RNE-rounded; error
    analysis gives ~6e-4 relative loss error, far under the 2e-2 gate).
  - int_sum: DoubleRow diagonal matmuls (+powers pairs) from fp8 true_sum
    planes accumulate +int_sum into the 4 main PSUM banks (warms the PE).
  - main matmuls: fp8e4 x fp8e4 DoubleRow, 256-deep contraction per MM:
    psum += intw.T @ latq = -pred  ->  psum = int_sum - pred = -diff.
  - loss: ACT Square+accum_out from PSUM -> partials [128, 4] per core.
  - head DMAs are chunked (w8 per mega-round, tq per plane-pair) so the PE
    and DVE start within ~2-3us instead of waiting for whole tensors.
"""

import numpy as np
import ml_dtypes

IN_FEATURES = 8192
OUT_FEATURES = 1024
N_BITS = 8
BATCH = 2048
N_CORES = 8
OPC = OUT_FEATURES // N_CORES  # 128 outputs per core
KP = 128                       # k per subtile (partition dim)
KT = IN_FEATURES // KP         # 64 k-subtiles
DKT = KT // 2                  # 32 DoubleRow k-tiles
NRND = 16                      # pack rounds (4 kt each)
MEGA = 4                       # unpack mega-rounds (4 rounds each)
NCHUNK = 512                   # moving free dim per matmul
NCH = BATCH // NCHUNK          # 4 batch chunks
# latent chunk schedule (kt per chunk), tapered tail
_LCH = [8, 8, 8, 8, 8, 8, 8, 4, 2, 2]
LCH_START = {}
_s = 0
for _i, _n in enumerate(_LCH):
    LCH_START[_s] = (_i, _n)
    _s += _n
assert _s == KT
POWERS = [1.0, 2.0, 4.0, 8.0, 16.0, 32.0, 64.0, -128.0]
# nibble-plane coefficients: psum = -int_w/1024, planes valued n*2^-9
C_PLANES = [-0.5, -8.0, 128.0]   # lo nibble, hi nibble, sign bit
SCALE = 2.0 ** N_BITS - 1.0

_CACHE: dict = {}


def _build():
    import concourse.bacc as bacc
    import concourse.mybir as mybir
    from concourse import tile

    f8e4 = mybir.dt.float8e4
    u8 = mybir.dt.uint8
    f32 = mybir.dt.float32
    Act = mybir.ActivationFunctionType
    Alu = mybir.AluOpType
    PM = mybir.MatmulPerfMode

    nc = bacc.Bacc("TRN2", target_bir_lowering=False, debug=False,
                   num_devices=N_CORES)

    latq = nc.dram_tensor("latq", [128, KT, BATCH], f8e4,
                          kind="ExternalInput")
    w8 = nc.dram_tensor("w8", [128, KT, OPC], u8, kind="ExternalInput")
    tq = nc.dram_tensor("tq", [OPC, N_BITS, BATCH], f8e4,
                        kind="ExternalInput")
    dg = nc.dram_tensor("dg", [OPC, N_BITS, OPC], f8e4,
                        kind="ExternalInput")
    cp = nc.dram_tensor("cp", [128, 3, 128], f8e4, kind="ExternalInput")
    partials = nc.dram_tensor("partials", [128, NCH], f32,
                              kind="ExternalOutput")

    with tile.TileContext(nc) as tc:
        with (
            tc.tile_pool(name="w8p", bufs=1) as w8_pool,
            tc.tile_pool(name="tsp", bufs=1) as tsp_pool,
            tc.tile_pool(name="cst", bufs=1) as cst_pool,
            tc.tile_pool(name="tp", bufs=2) as t_pool,
            tc.tile_pool(name="iw", bufs=1) as iw_pool,
            tc.tile_pool(name="lat", bufs=6) as lat_pool,
            tc.tile_pool(name="loss", bufs=1) as loss_pool,
            tc.tile_pool(name="ps", bufs=1, space="PSUM") as psum_pool,
            tc.tile_pool(name="pk", bufs=2, space="PSUM") as pk_pool,
        ):
            # ---- tiny constants first, then chunked heads ----
            dgt = cst_pool.tile([OPC, N_BITS, OPC], f8e4, name="dgt",
                                tag="dgt")
            nc.sync.dma_start(dgt[:], dg[:])
            cpt = cst_pool.tile([128, 3, 128], f8e4, name="cpt", tag="cpt")
            nc.sync.dma_start(cpt[:], cp[:])

            w8t = w8_pool.tile([128, KT, OPC], u8)
            tp = tsp_pool.tile([OPC, N_BITS, BATCH], f8e4)
            for h in range(4):
                nc.sync.dma_start(w8t[:, 16 * h:16 * (h + 1), :],
                                  w8[:, 16 * h:16 * (h + 1), :])
                nc.sync.dma_start(tp[:, 2 * h:2 * h + 2, :],
                                  tq[:, 2 * h:2 * h + 2, :])

            # ---- int_sum: psum[o, n] = +int_sum (DoubleRow diag MMs;
            # also warms the PE) ----
            psums = [psum_pool.tile([128, NCHUNK], f32, name=f"ps{i}",
                                    tag=f"ps{i}") for i in range(NCH)]
            for bp in range(4):
                for c in range(NCH):
                    nc.tensor.matmul(
                        psums[c][:],
                        dgt[:, 2 * bp:2 * bp + 2, :],
                        tp[:, 2 * bp:2 * bp + 2,
                           c * NCHUNK:(c + 1) * NCHUNK],
                        start=(bp == 0), stop=False,
                        perf_mode=PM.DoubleRow)

            # ---- weight pipeline + main matmul stream ----
            iwts = [iw_pool.tile([128, 4, OPC], f8e4, name=f"iw{r}",
                                 tag=f"iw{r}") for r in range(NRND)]
            out_t = loss_pool.tile([128, NCH], f32, name="out_t",
                                   tag="out_t")
            lts = {}

            def issue_lat(kt):
                if kt in LCH_START:
                    q, n = LCH_START[kt]
                    lt = lat_pool.tile([128, n, BATCH], f8e4,
                                       name=f"lt{q}", tag="lat")
                    nc.sync.dma_start(lt[:], latq[:, kt:kt + n, :])
                    lts[kt] = (lt, kt)

            for mr in range(MEGA):
                # unpack 16 kt worth: 3 nibble planes, t[q, pl, 16*OPC]
                tt = t_pool.tile([128, 3, 16 * OPC], f8e4,
                                 name=f"tt{mr}", tag="tt")
                w8s = w8t[:, 16 * mr:16 * (mr + 1), :]
                nc.vector.tensor_scalar(tt[:, 0, :].bitcast(u8), w8s,
                                        15, None, Alu.bitwise_and)
                nc.vector.tensor_scalar(tt[:, 1, :].bitcast(u8), w8s,
                                        4, 15, Alu.logical_shift_right,
                                        Alu.bitwise_and)
                nc.vector.tensor_scalar(tt[:, 2, :].bitcast(u8), w8s,
                                        7, None, Alu.logical_shift_right)

                for rr in range(4):          # rounds within mega-round
                    r = 4 * mr + rr
                    # pack: pkb = -int_w/1024 for 4 kt (512 cols)
                    pkb = pk_pool.tile([128, NCHUNK], f32, name=f"pk{r}",
                                       tag="pk")
                    nc.tensor.matmul(
                        pkb[:], cpt[:, 0:2, :],
                        tt[:, 0:2, rr * NCHUNK:(rr + 1) * NCHUNK],
                        start=True, stop=False, perf_mode=PM.DoubleRow)
                    nc.tensor.matmul(
                        pkb[:], cpt[:, 2, :],
                        tt[:, 2, rr * NCHUNK:(rr + 1) * NCHUNK],
                        start=False, stop=True)
                    # cast: intw = -int_w (fp8e4, RNE)
                    nc.scalar.activation(iwts[r][:], pkb[:], Act.Copy,
                                         scale=1024.0)
                    # main MMs for dkt = 2r, 2r+1
                    for i in range(2):
                        dkt = 2 * r + i
                        kt0 = 2 * dkt
                        issue_lat(kt0)
                        cur = lts[max(s for s in lts if s <= kt0)]
                        lt, base = cur
                        a = kt0 - base
                        lhsT = iwts[r][:, 2 * i:2 * i + 2, :]
                        last = (dkt == DKT - 1)
                        for c in range(NCH):
                            nc.tensor.matmul(
                                psums[c][:], lhsT,
                                lt[:, a:a + 2,
                                   c * NCHUNK:(c + 1) * NCHUNK],
                                start=False, stop=last,
                                perf_mode=PM.DoubleRow)
                            if last:
                                # interleave loss squares with final MMs
                                d2 = loss_pool.tile(
                                    [128, NCHUNK], f32, name=f"d2_{c}",
                                    tag=f"d2_{c}")
                                nc.scalar.activation(
                                    d2[:], psums[c][:], Act.Square,
                                    accum_out=out_t[:, c:c + 1])

            nc.sync.dma_start(partials[:], out_t[:])

    nc.compile()
    return nc


def _get_nc():
    if "nc" not in _CACHE:
        _CACHE["nc"] = _build()
    return _CACHE["nc"]


def make_in_maps(latent: np.ndarray, true_sum: np.ndarray,
                 weight: np.ndarray) -> list:
    f8 = ml_dtypes.float8_e4m3fn

    # latq[p, kt, n] = latent[n, kt*128 + p]
    lat8 = latent.astype(f8)
    latq = np.ascontiguousarray(
        lat8.T.reshape(KT, KP, BATCH).transpose(1, 0, 2))

    # sign bits, packed 8 per byte: byte[k, o] bit b = (weight[k, o*8+b] > 0)
    bits = (weight > 0).astype(np.uint8).reshape(IN_FEATURES,
                                                 OUT_FEATURES, N_BITS)
    shifts = (1 << np.arange(N_BITS, dtype=np.uint16))
    bytes_ko = (bits.astype(np.uint16) * shifts).sum(-1).astype(np.uint8)

    # diag pairs for int_sum: dg[o, b, o'] = powers[b] * (o == o')
    dg = np.zeros((OPC, N_BITS, OPC), dtype=np.float32)
    for b in range(N_BITS):
        np.fill_diagonal(dg[:, b, :], POWERS[b])
    dg8 = dg.astype(f8)

    # pack plane coefficients
    cpm = np.zeros((128, 3, 128), dtype=np.float32)
    for j in range(3):
        np.fill_diagonal(cpm[:, j, :], C_PLANES[j])
    cp8 = cpm.astype(f8)

    in_maps = []
    for c in range(N_CORES):
        # w8[q, kt, o] = bytes_ko[kt*128+q, c*128+o]
        wcol = bytes_ko[:, c * OPC:(c + 1) * OPC]
        w8 = np.ascontiguousarray(
            wcol.reshape(KT, KP, OPC).transpose(1, 0, 2))
        T = true_sum[:, c * OPC * N_BITS:(c + 1) * OPC * N_BITS]
        tql = np.ascontiguousarray(
            T.reshape(BATCH, OPC, N_BITS).transpose(1, 2, 0)).astype(f8)
        in_maps.append({"latq": latq, "w8": w8, "tq": tql,
                        "dg": dg8, "cp": cp8})
    return in_maps


def kernel(latent: np.ndarray, true_sum: np.ndarray,
           weight: np.ndarray) -> np.ndarray:
    from concourse.bass_utils import run_bass_kernel_spmd

    nc = _get_nc()
    in_maps = make_in_maps(latent, true_sum, weight)
    res = run_bass_kernel_spmd(nc, in_maps, list(range(N_CORES)))

    total = 0.0
    for c in range(N_CORES):
        total += float(res.results[c]["partials"].astype(np.float64).sum())
    loss = total / (BATCH * OUT_FEATURES) / (SCALE * SCALE)
    return np.array(loss, dtype=np.float32)


# revision 5
# speedup vs baseline: 1.1065x; 1.0717x over previous
"""Trainium2 Bass kernel for BinaryDecoderV2 — v3 (2x4 sharding).

Same pipeline as v2.5 (bit-packed weights, 3-pass nibble unpack, diagonal
pack matmuls, fp8 DoubleRow mains) but sharded 2-way over batch x 4-way over
out_features: per-core latent halves to 8.4MB, weights bits 2.1MB, true_sum
2.1MB -> ~12.7MB HBM/core instead of 20.2MB. PE main work is unchanged
(same MACs/core); pack work doubles (weight slice is 2x wider) but stays
far under the DMA roofline.

Core c: out-shard c%4 (256 outputs), batch-shard c//4 (1024 rows).
"""

import numpy as np
import ml_dtypes

IN_FEATURES = 8192
OUT_FEATURES = 1024
N_BITS = 8
BATCH = 2048
N_CORES = 8
OSH = 4                        # out-feature shards
BSH = 2                        # batch shards
OPC = OUT_FEATURES // OSH      # 256 outputs per core
BC = BATCH // BSH              # 1024 batch rows per core
KP = 128
KT = IN_FEATURES // KP         # 64 k-subtiles
DKT = KT // 2                  # 32 DoubleRow k-tiles
NRND = 32                      # pack rounds (2 kt each)
MEGA = 4                       # unpack mega-rounds (16 kt each)
NCHUNK = 512
NCH = BC // NCHUNK             # 2 batch chunks per core
_LCH = [8, 8, 8, 8, 8, 8, 8, 4, 2, 2]
LCH_START = {}
_s = 0
for _i, _n in enumerate(_LCH):
    LCH_START[_s] = (_i, _n)
    _s += _n
assert _s == KT
POWERS = [1.0, 2.0, 4.0, 8.0, 16.0, 32.0, 64.0, -128.0]
C_PLANES = [-0.5, -8.0]   # lo nibble, hi (sign-flipped) nibble
SCALE = 2.0 ** N_BITS - 1.0

_CACHE: dict = {}


def _build():
    import concourse.bacc as bacc
    import concourse.mybir as mybir
    from concourse import tile

    f8e4 = mybir.dt.float8e4
    u8 = mybir.dt.uint8
    f32 = mybir.dt.float32
    Act = mybir.ActivationFunctionType
    Alu = mybir.AluOpType
    PM = mybir.MatmulPerfMode

    nc = bacc.Bacc("TRN2", target_bir_lowering=False, debug=False,
                   num_devices=N_CORES)

    latq = nc.dram_tensor("latq", [128, KT, BC], f8e4,
                          kind="ExternalInput")
    w8 = nc.dram_tensor("w8", [128, KT, OPC], u8, kind="ExternalInput")
    tq = nc.dram_tensor("tq", [128, 2, N_BITS, BC], f8e4,
                        kind="ExternalInput")
    dg = nc.dram_tensor("dg", [128, N_BITS, 128], f8e4,
                        kind="ExternalInput")
    cp = nc.dram_tensor("cp", [128, 2, 128], f8e4, kind="ExternalInput")
    partials = nc.dram_tensor("partials", [128, 4], f32,
                              kind="ExternalOutput")

    with tile.TileContext(nc) as tc:
        with (
            tc.tile_pool(name="w8p", bufs=1) as w8_pool,
            tc.tile_pool(name="tsp", bufs=1) as tsp_pool,
            tc.tile_pool(name="cst", bufs=1) as cst_pool,
            tc.tile_pool(name="tp", bufs=2) as t_pool,
            tc.tile_pool(name="iw", bufs=1) as iw_pool,
            tc.tile_pool(name="lat", bufs=6) as lat_pool,
            tc.tile_pool(name="loss", bufs=1) as loss_pool,
            tc.tile_pool(name="ps", bufs=1, space="PSUM") as psum_pool,
            tc.tile_pool(name="pk", bufs=2, space="PSUM") as pk_pool,
        ):
            dgt = cst_pool.tile([128, N_BITS, 128], f8e4, name="dgt",
                                tag="dgt")
            nc.sync.dma_start(dgt[:], dg[:])
            cpt = cst_pool.tile([128, 2, 128], f8e4, name="cpt", tag="cpt")
            nc.sync.dma_start(cpt[:], cp[:])

            w8t = w8_pool.tile([128, KT, OPC], u8)
            tp = tsp_pool.tile([128, 2, N_BITS, BC], f8e4)
            for h in range(4):
                nc.sync.dma_start(w8t[:, 16 * h:16 * (h + 1), :],
                                  w8[:, 16 * h:16 * (h + 1), :])
                nc.sync.dma_start(tp[:, :, 2 * h:2 * h + 2, :],
                                  tq[:, :, 2 * h:2 * h + 2, :])

            # ---- int_sum into 4 psums: index = oh*2 + ch ----
            psums = [psum_pool.tile([128, NCHUNK], f32, name=f"ps{i}",
                                    tag=f"ps{i}") for i in range(4)]
            for bp in range(4):
                for oh in range(2):
                    for ch in range(NCH):
                        nc.tensor.matmul(
                            psums[oh * NCH + ch][:],
                            dgt[:, 2 * bp:2 * bp + 2, :],
                            tp[:, oh, 2 * bp:2 * bp + 2,
                               ch * NCHUNK:(ch + 1) * NCHUNK],
                            start=(bp == 0), stop=False,
                            perf_mode=PM.DoubleRow)

            # ---- weight pipeline + main matmul stream ----
            iwts = [iw_pool.tile([128, 2, OPC], f8e4, name=f"iw{r}",
                                 tag=f"iw{r}") for r in range(NRND)]
            out_t = loss_pool.tile([128, 4], f32, name="out_t",
                                   tag="out_t")
            lts = {}

            def issue_lat(kt):
                if kt in LCH_START:
                    q, n = LCH_START[kt]
                    lt = lat_pool.tile([128, n, BC], f8e4,
                                       name=f"lt{q}", tag="lat")
                    nc.sync.dma_start(lt[:], latq[:, kt:kt + n, :])
                    lts[kt] = (lt, kt)

            for mr in range(MEGA):
                # unpack 16 kt worth: 2 nibble planes [128, 2, 16*OPC]
                # (host pre-XORs 0x80, so hi' = x>>4 and the -128 constant
                # folds into the cast bias)
                tt = t_pool.tile([128, 2, 16 * OPC], f8e4,
                                 name=f"tt{mr}", tag="tt")
                w8s = w8t[:, 16 * mr:16 * (mr + 1), :]
                nc.vector.tensor_scalar(tt[:, 0, :].bitcast(u8), w8s,
                                        15, None, Alu.bitwise_and)
                nc.vector.tensor_scalar(tt[:, 1, :].bitcast(u8), w8s,
                                        4, None, Alu.logical_shift_right)

                for rr in range(8):          # rounds (2 kt) in mega-round
                    r = 8 * mr + rr
                    pkb = pk_pool.tile([128, NCHUNK], f32, name=f"pk{r}",
                                       tag="pk")
                    nc.tensor.matmul(
                        pkb[:], cpt[:, 0:2, :],
                        tt[:, 0:2, rr * NCHUNK:(rr + 1) * NCHUNK],
                        start=True, stop=True, perf_mode=PM.DoubleRow)
                    # intw = 1024*(-(n_lo+16*n_hi')/1024) + 128 = -int_w
                    nc.scalar.activation(iwts[r][:], pkb[:], Act.Copy,
                                         scale=1024.0, bias=128.0)
                    # main MMs for dkt = r (one DoubleRow k-tile per round)
                    dkt = r
                    kt0 = 2 * dkt
                    issue_lat(kt0)
                    cur = lts[max(s for s in lts if s <= kt0)]
                    lt, base = cur
                    a = kt0 - base
                    last = (dkt == DKT - 1)
                    for oh in range(2):
                        lhsT = iwts[r][:, :, oh * 128:(oh + 1) * 128]
                        for ch in range(NCH):
                            nc.tensor.matmul(
                                psums[oh * NCH + ch][:], lhsT,
                                lt[:, a:a + 2,
                                   ch * NCHUNK:(ch + 1) * NCHUNK],
                                start=False, stop=last,
                                perf_mode=PM.DoubleRow)
                            if last:
                                i4 = oh * NCH + ch
                                d2 = loss_pool.tile(
                                    [128, NCHUNK], f32, name=f"d2_{i4}",
                                    tag=f"d2_{i4}")
                                nc.scalar.activation(
                                    d2[:], psums[i4][:], Act.Square,
                                    accum_out=out_t[:, i4:i4 + 1])

            nc.sync.dma_start(partials[:], out_t[:])

    nc.compile()
    return nc


def _get_nc():
    if "nc" not in _CACHE:
        _CACHE["nc"] = _build()
    return _CACHE["nc"]


def make_in_maps(latent: np.ndarray, true_sum: np.ndarray,
                 weight: np.ndarray) -> list:
    f8 = ml_dtypes.float8_e4m3fn

    # latq per batch shard: latq[p, kt, n] = latent[sb*BC + n, kt*128 + p]
    lat8 = latent.astype(f8)
    latqs = []
    for sb in range(BSH):
        ls = lat8[sb * BC:(sb + 1) * BC, :]
        latqs.append(np.ascontiguousarray(
            ls.T.reshape(KT, KP, BC).transpose(1, 0, 2)))

    bits = (weight > 0).astype(np.uint8).reshape(IN_FEATURES,
                                                 OUT_FEATURES, N_BITS)
    shifts = (1 << np.arange(N_BITS, dtype=np.uint16))
    bytes_ko = ((bits.astype(np.uint16) * shifts).sum(-1)
                .astype(np.uint8) ^ 0x80)   # flip sign bit: -128 -> cast bias
    w8s = []
    for so in range(OSH):
        wcol = bytes_ko[:, so * OPC:(so + 1) * OPC]
        w8s.append(np.ascontiguousarray(
            wcol.reshape(KT, KP, OPC).transpose(1, 0, 2)))

    dg = np.zeros((128, N_BITS, 128), dtype=np.float32)
    for b in range(N_BITS):
        np.fill_diagonal(dg[:, b, :], POWERS[b])
    dg8 = dg.astype(f8)
    cpm = np.zeros((128, 2, 128), dtype=np.float32)
    for j in range(2):
        np.fill_diagonal(cpm[:, j, :], C_PLANES[j])
    cp8 = cpm.astype(f8)

    ts8 = true_sum.astype(f8)
    in_maps = []
    for c in range(N_CORES):
        so, sb = c % OSH, c // OSH
        # tq[o128, oh, b, n] = true_sum[sb*BC+n, (so*256 + oh*128 + o128)*8 + b]
        T = ts8[sb * BC:(sb + 1) * BC,
                so * OPC * N_BITS:(so + 1) * OPC * N_BITS]
        t5 = T.reshape(BC, 2, 128, N_BITS)       # [n, oh, o128, b]
        tql = np.ascontiguousarray(t5.transpose(2, 1, 3, 0))
        in_maps.append({"latq": latqs[sb], "w8": w8s[so], "tq": tql,
                        "dg": dg8, "cp": cp8})
    return in_maps


def kernel(latent: np.ndarray, true_sum: np.ndarray,
           weight: np.ndarray) -> np.ndarray:
    from concourse.bass_utils import run_bass_kernel_spmd

    nc = _get_nc()
    in_maps = make_in_maps(latent, true_sum, weight)
    res = run_bass_kernel_spmd(nc, in_maps, list(range(N_CORES)))

    total = 0.0
    for c in range(N_CORES):
        total += float(res.results[c]["partials"].astype(np.float64).sum())
    loss = total / (BATCH * OUT_FEATURES) / (SCALE * SCALE)
    return np.array(loss, dtype=np.float32)
